# revision 44
# baseline (speedup 1.0000x reference)
"""Distributed masked-attention kernel for one TRN2 chip (8 NeuronCores).

Problem: B=4, S=4096, IN=512, D=64 attention with a [S,S] int32 score mask
(masked scores replaced by 1e-6 *before* softmax, so masked probs are
exp(1e-6)/Z ~= 1/Z, NOT zero).

Sharding (8 cores): core c = b*2 + qh -> batch b in {0..3}, query rows
[2048*qh, 2048*(qh+1)). Inputs are rolled along S so the core's own query
slab is at rows [0:2048) (attention's k-sum is permutation invariant) ->
all 8 cores run the IDENTICAL graph (SPMD).

The steady-state loop is DVE-bound on TRN2: the mask multiply must read the
f32 PSUM scores at 1x (bf16 PSUM matmul output that would enable the DVE
2x_1p mode is TRN3-only), so 64 x [128,1024] tensor_tensor ops ~= 78us is
the floor. Everything else is arranged to hide under it:
  PE:  S^T = (K^T block)^T @ Q^T  (bf16, Q^T zero-padded to 128 partitions
       -> full-array contraction keeps the PE HAM at 2.4GHz)
  DVE: sm = S^T * mask -> SBUF bf16 (1x from f32 PSUM: the pacing stream).
       Mid-loop KV bias-adds/vaug copies also ride DVE (prompt) instead of
       parking behind a 3.7us exp on the scalar queue.
  ACT: P = exp(0.125*sm) in FD=4096 groups (2 k-tiles x 2 q-halves per
       ACTIVATE) -> ~59us total, never paces the loop.
  PE:  O^T[65, q] += V_aug^T @ P  (V_aug = [V | 1]: ones column emits the
       softmax denominator for free)

Ramp: every DMA rides the sync ring in strict deadline order (the sem-lane
reuse parking then lands on the idle sync queue, not the scalar engine);
~7 warm-up matmuls on scratch data unthrottle the PE HAM before the real
projections; the exp table loads at t~7us via a dummy ACTIVATE; the qh1
Q-projection is interleaved between the first k-tile's score halves.

Epilogue per q-half: two 512-col sub-blocks, each PSUM->SBUF copy (ACT),
4 PE transposes, strided reciprocal of the denominator row (DVE), divides
split DVE/ACT, store -- sub-block 0 starts as soon as its own PV
accumulation chain stops, pipelining the tail across engines.
"""

import sys

if "/opt/trn_rl_repo" not in sys.path:
    sys.path.insert(0, "/opt/trn_rl_repo")

from contextlib import ExitStack

import numpy as np

import concourse.bass as bass
import concourse.bacc as bacc
import concourse.mybir as mybir
import concourse.tile as tile
from concourse.bass_utils import run_bass_kernel_spmd
from concourse.masks import make_identity

ts = bass.ts
ds = bass.ds

N_CORES = 8
B, S, C, D = 4, 4096, 512, 64
Q_LOC = 2048       # query rows per core
QH = 1024          # one q-half (pipeline stream) width
N_KT = S // 128    # 32 k-tiles of 128
QC = 512           # matmul moving chunk
N_MG = 8           # mask DMA groups (4 k-tiles each)

F32 = mybir.dt.float32
BF16 = mybir.dt.bfloat16
I8 = mybir.dt.int8
AF = mybir.ActivationFunctionType
ALU = mybir.AluOpType

# exp groups: [kt0] alone (early ACT start), then pairs, then singles at the
# end (short tail + lets the PV lag taper off); kt31 is handled separately.
GROUPS = [[0]] + [[1 + 2 * i, 2 + 2 * i] for i in range(14)] + [[29], [30]]
# remaining KV chunk-halves (c,h): the projection (4 matmuls + bias) and
# the V-transposes (+vaug copy) are emitted at ADJACENT groups' starts,
# batched in pairs -- two smaller insertions stall the in-order PE queue
# (and so the DVE mask-multiply stream) less than one big one, and the
# deferral is safe because kvt is stable SBUF and vaug is only read by the
# lagged PV.  Measured: split 122.8us vs fused 123.9 vs singles 123.8+.
KV_SCHED_P = {
    1: [(0, 1), (1, 0)],
    5: [(1, 1), (2, 0)],
    9: [(2, 1), (3, 0)],
    12: [(3, 1)],
}
KV_SCHED_T = {
    2: [(0, 1), (1, 0)],
    6: [(1, 1), (2, 0)],
    10: [(2, 1), (3, 0)],
    13: [(3, 1)],
}


def build_kernel() -> bacc.Bacc:
    nc = bacc.Bacc(None, target_bir_lowering=False, debug=False)

    xt_ext = nc.declare_dram_parameter("xt", [4, 128, 4, 1024], BF16, isOutput=False)
    mt_ext = nc.declare_dram_parameter("maskp", [N_MG, 128, 4 * Q_LOC], I8, isOutput=False)
    # packed weights: [Wk | Wv | Wq] along the last dim
    w_ext = nc.declare_dram_parameter("wqkv", [128, 4, 3 * D], BF16, isOutput=False)
    # packed biases: col0 = [bk; bv], col1 = [bq; 0]
    b_ext = nc.declare_dram_parameter("biases", [128, 2], F32, isOutput=False)
    out_ext = nc.declare_dram_parameter("out", [Q_LOC, D], F32, isOutput=True)

    with tile.TileContext(nc) as tc, ExitStack() as ctx:
        # ---------------- pools ----------------
        persist = ctx.enter_context(tc.tile_pool(name="persist", bufs=1))
        sm_pool = ctx.enter_context(tc.tile_pool(name="sm", bufs=4))
        pt_pool = ctx.enter_context(tc.tile_pool(name="pt", bufs=5))
        epi = ctx.enter_context(tc.tile_pool(name="epi", bufs=1))
        epi2 = ctx.enter_context(tc.tile_pool(name="epi2", bufs=2))
        psum_s = ctx.enter_context(
            tc.tile_pool(name="psum_s", bufs=2, space=bass.MemorySpace.PSUM)
        )
        psum_o = ctx.enter_context(
            tc.tile_pool(name="psum_o", bufs=1, space=bass.MemorySpace.PSUM)
        )

        # ---------------- persistent tiles ----------------
        wt = persist.tile([128, 4, 3 * D], BF16)
        biasT = persist.tile([128, 2], F32)
        xb = persist.tile([128, 4, 4, 1024], BF16, name="xb", tag="xb")
        mg = [persist.tile([128, 4, Q_LOC], I8, name=f"mg{g}", tag=f"mg{g}") for g in range(N_MG)]
        kvt = persist.tile([128, S], BF16, name="kvt", tag="kvt")
        qt_t = persist.tile([128, Q_LOC], BF16, name="qt", tag="qt")
        vaug = persist.tile([128, N_KT, D + 1], BF16, name="va", tag="va")
        scr_in = persist.tile([128, 8], BF16, name="scr_in", tag="scr")
        scr_out = persist.tile([128, 8], BF16, name="scr_out", tag="scr2")
        # init memsets on the (ramp-idle) DVE
        nc.vector.memset(qt_t[D:128, :], 0.0)
        nc.vector.memset(vaug[:, :, D : D + 1], 1.0)
        nc.vector.memset(scr_in[:], 0.0)
        # scratch source for PE warm-up matmuls (HAM needs ~3.4us of activity
        # before it unthrottles 1.2 -> 2.4 GHz)
        nc.vector.memset(kvt[:, 0:QC], 0.0)

        # Preload the exp table set right away (one-time ~2.7us); the scalar
        # queue carries no DMA triggers so this runs at t~7us.
        nc.scalar.activation(scr_out[:], scr_in[:], AF.Exp)

        # ---------------- DMAs ----------------

        # ALL DMAs ride the sync ring in strict deadline order: a single ring
        # gets full SDMA bandwidth per transfer (FIFO, no cross-ring
        # round-robin), and the unavoidable semaphore-lane-reuse parking of
        # later triggers lands on the otherwise idle sync queue instead of
        # blocking the scalar engine.
        def dma_m(g):
            nc.sync.dma_start(mg[g][:].rearrange("p t q -> p (t q)"), mt_ext[g])

        def dma_x(cc):
            # x host-packed [cc, p, j, 1024]: each partition's chunk slice is
            # 8KB contiguous = one descriptor (full DMA rate; the column-
            # sliced view was descriptor-rate-bound at ~166GB/s)
            nc.sync.dma_start(
                xb[:, cc, :, :].rearrange("p j s -> p (j s)"), xt_ext[cc]
            )

        dma_x(0)
        nc.sync.dma_start(wt[:], w_ext[:])
        nc.sync.dma_start(biasT[:], b_ext[:])
        dma_m(0)
        dma_x(1)
        dma_m(1)
        dma_x(2)
        dma_x(3)
        for g in range(2, N_MG):
            dma_m(g)

        # ---------------- constants ----------------
        ident_f = persist.tile([128, 128], F32)
        make_identity(nc, ident_f[:])
        ident_b = persist.tile([128, 128], BF16)
        make_identity(nc, ident_b[:])

        def emit_kv_proj(c: int, h: int, on_dve: bool = False):
            # on_dve: mid-loop the bias-apply/copy run on the (bottleneck but
            # prompt) DVE queue -- on ACT they'd sit behind a 3.7us exp and
            # starve the shared PSUM pool's next score tile.
            kv_ps = psum_s.tile([128, QC], F32, name="kvps", tag="ps")
            for j in range(4):
                nc.tensor.matmul(
                    kv_ps[:],
                    wt[:, j, 0 : 2 * D],
                    xb[:, c, j, ds(h * QC, QC)],
                    start=(j == 0),
                    stop=(j == 3),
                )
            if on_dve:
                nc.vector.tensor_scalar(
                    kvt[:, ds(c * 1024 + h * QC, QC)], kv_ps[:],
                    biasT[:, 0:1], None, op0=ALU.add,
                )
            else:
                nc.scalar.activation(
                    kvt[:, ds(c * 1024 + h * QC, QC)], kv_ps[:], AF.Identity,
                    bias=biasT[:, 0:1],
                )

        def emit_kv_tran(c: int, h: int, on_dve: bool = False):
            # V^T -> vaug transposes, deferred one group after the projection
            # (kvt is stable SBUF; vaug[kt] is only read by the lagged PV)
            vp = psum_s.tile([128, 4, D], BF16, name="vp", tag="ps")
            kt0 = 8 * c + 4 * h
            for u in range(4):
                nc.tensor.transpose(
                    vp[:, u, :],
                    kvt[D : 2 * D, ts(kt0 + u, 128)],
                    ident_b[D : 2 * D, D : 2 * D],
                )
            if on_dve:
                nc.vector.tensor_copy(vaug[:, kt0 : kt0 + 4, 0:D], vp[:])
            else:
                nc.scalar.copy(vaug[:, kt0 : kt0 + 4, 0:D], vp[:])

        def emit_kv_half(c: int, h: int, on_dve: bool = False):
            emit_kv_proj(c, h, on_dve)
            emit_kv_tran(c, h, on_dve)

        def emit_q(qh: int):
            q_ps = psum_s.tile([D, QH], F32, name="qps", tag="ps")
            for h in range(QH // QC):
                for j in range(4):
                    nc.tensor.matmul(
                        q_ps[:, ts(h, QC)],
                        wt[:, j, 2 * D : 3 * D],
                        xb[:, qh, j, ds(h * QC, QC)],
                        start=(j == 0),
                        stop=(j == 3),
                    )
            nc.scalar.activation(
                qt_t[0:D, ts(qh, QH)], q_ps[:], AF.Identity, bias=biasT[0:D, 1:2]
            )

        def emit_scores_tt(kt: int, qh: int, sm_q):
            """scores for (kt, qh) -> PSUM f32 -> masked bf16 into sm_q."""
            st = psum_s.tile([128, QH], F32, name="st", tag="ps")
            for qc in range(QH // QC):
                nc.tensor.matmul(
                    st[:, ts(qc, QC)],
                    kvt[:, ts(kt, 128)],
                    qt_t[:, ds(qh * QH + qc * QC, QC)],
                    start=True,
                    stop=True,
                )
            mk = mg[kt // 4][:, kt % 4, ts(qh, QH)]
            nc.vector.tensor_tensor(out=sm_q, in0=st[:], in1=mk, op=ALU.mult)

        def emit_pv(qh, kt, ot, pt_q, first, last):
            for qc in range(QH // QC):
                nc.tensor.matmul(
                    ot[:, ds(qh * QH + qc * QC, QC)],
                    vaug[:, kt, :],
                    pt_q[:, ts(qc, QC)],
                    start=first,
                    stop=last,
                )

        def emit_epilogue_half(ot, half):
            # one q-half, processed as two 512-col sub-blocks that pipeline
            # across ACT (copy) / PE (transpose) / DVE (recip, divide): the
            # first sub-block only depends on its own PV accumulation chain,
            # so it starts while the second is still accumulating.
            oext = out_ext[:].rearrange("(hf qt p) d -> hf p qt d", hf=2, p=128)
            rcp = epi2.tile([128, 16], F32, tag="rcp")
            for sub in range(2):
                ots = epi.tile([D + 1, QC], F32, tag=f"ots{half}{sub}")
                nc.scalar.copy(ots[:], ot[:, ds(half * QH + sub * QC, QC)])
                of = epi2.tile([128, 4, D], F32, tag=f"of{half}{sub}")
                op8 = psum_s.tile([128, 4, 128], F32, name="op8", tag="ps")
                for i in range(4):
                    nc.tensor.transpose(
                        op8[:, i, 0 : D + 1], ots[:, ts(i, 128)],
                        ident_f[0 : D + 1, 0 : D + 1],
                    )
                r0 = 8 * half + 4 * sub
                nc.vector.reciprocal(
                    rcp[:, ds(r0, 4)], op8[:, :, D : D + 1].rearrange("p t o -> p (t o)")
                )
                for i in range(4):
                    # divides split across DVE and ACT -- both idle in the tail
                    r = rcp[:, r0 + i : r0 + i + 1]
                    if i % 2 == 0:
                        nc.vector.tensor_scalar(
                            of[:, i, :], op8[:, i, 0:D], r, None, op0=ALU.mult
                        )
                    else:
                        nc.scalar.activation(
                            of[:, i, :], op8[:, i, 0:D], AF.Copy, scale=r
                        )
                nc.sync.dma_start(oext[half][:, ds(4 * sub, 4), :], of[:])

        # ---------------- emission ----------------
        ot = psum_o.tile([D + 1, Q_LOC], F32, name="ot", tag="ot")
        # PE warm-up: dummy matmuls on scratch data while x streams in, so
        # the HAM unthrottles before the real projections start.
        wrm = psum_s.tile([128, QC], F32, name="wrm", tag="ps")
        for _ in range(5):
            nc.tensor.matmul(wrm[:], ident_b[:], kvt[:, 0:QC], start=True, stop=True)
        emit_kv_half(0, 0)
        emit_q(0)

        pv_queue = []  # (kt, pt_qh0, pt_qh1)

        def drain_pv(keep):
            # pop entries while more than `keep` remain queued.  The kept
            # backlog is the PV lag: a popped entry's exp must be long done
            # or the in-order PE queue head-of-line blocks on it.
            while len(pv_queue) > keep:
                kt, pa, pb = pv_queue.pop(0)
                emit_pv(0, kt, ot, pa, kt == 0, False)
                emit_pv(1, kt, ot, pb, kt == 0, False)

        def emit_exp(sm, pt):
            nc.scalar.activation(
                pt[:].rearrange("p a q -> p (a q)"),
                sm[:].rearrange("p a q -> p (a q)"),
                AF.Exp,
                scale=0.125,
            )

        # group 0 = [kt0]: interleave the qh1 Q-projection between the two
        # score halves so the first TT fires as soon as q(0)+mask g0 land.
        sm = sm_pool.tile([128, 2, QH], BF16, tag="sm")
        emit_scores_tt(0, 0, sm[:, 0, :])
        emit_q(1)
        emit_scores_tt(0, 1, sm[:, 1, :])
        pt = pt_pool.tile([128, 2, QH], BF16, tag="pt")
        emit_exp(sm, pt)
        pv_queue.append((0, pt[:, 0, :], pt[:, 1, :]))

        n_groups = len(GROUPS) + 1  # +1: kt31 handled below
        for gi, kts in list(enumerate(GROUPS))[1:]:
            for ch in KV_SCHED_P.get(gi, []):
                emit_kv_proj(*ch, on_dve=True)
            for ch in KV_SCHED_T.get(gi, []):
                emit_kv_tran(*ch, on_dve=True)
            # taper the lag near the end so the tail flush stays short
            keep = 5 if gi < n_groups - 6 else (3 if gi < n_groups - 4 else (2 if gi < n_groups - 2 else 1))
            nq = 2 * len(kts)
            sm = sm_pool.tile([128, nq, QH], BF16, tag="sm")
            for i, kt in enumerate(kts):
                for qh in range(2):
                    emit_scores_tt(kt, qh, sm[:, 2 * i + qh, :])
                drain_pv(keep)
            pt = pt_pool.tile([128, nq, QH], BF16, tag="pt")
            emit_exp(sm, pt)
            for i, kt in enumerate(kts):
                pv_queue.append((kt, pt[:, 2 * i, :], pt[:, 2 * i + 1, :]))

        # kt31: per-qh FD=1024 exps so the qh0 PV + epilogue start ~1us
        # earlier, and the PV flush + both epilogue halves pipeline across
        # PE/ACT/DVE.
        sm31 = sm_pool.tile([128, 2, QH], BF16, tag="sm")
        pt31 = pt_pool.tile([128, 2, QH], BF16, tag="pt")
        emit_scores_tt(31, 0, sm31[:, 0, :])
        nc.scalar.activation(pt31[:, 0, :], sm31[:, 0, :], AF.Exp, scale=0.125)
        emit_scores_tt(31, 1, sm31[:, 1, :])
        nc.scalar.activation(pt31[:, 1, :], sm31[:, 1, :], AF.Exp, scale=0.125)
        drain_pv(0)
        emit_pv(0, 31, ot, pt31[:, 0, :], False, True)
        emit_pv(1, 31, ot, pt31[:, 1, :], False, True)
        emit_epilogue_half(ot, 0)
        emit_epilogue_half(ot, 1)

    nc.compile()
    return nc


def _shard_inputs(input_embedding, mask, Wq, bq, Wk, bk, Wv, bv):
    import ml_dtypes

    input_embedding = np.asarray(input_embedding, dtype=np.float32)
    mask = np.asarray(mask, dtype=np.int32)

    def pack_w(w):
        return np.ascontiguousarray(
            np.asarray(w, np.float32).reshape(4, 128, -1).transpose(1, 0, 2)
        ).astype(ml_dtypes.bfloat16)

    wqkv = np.concatenate([pack_w(Wk), pack_w(Wv), pack_w(Wq)], axis=2)
    biases = np.zeros((128, 2), np.float32)
    biases[:, 0] = np.concatenate([np.asarray(bk, np.float32), np.asarray(bv, np.float32)])
    biases[0:D, 1] = np.asarray(bq, np.float32)
    w = {
        "wqkv": np.ascontiguousarray(wqkv),
        "biases": np.ascontiguousarray(biases),
    }
    in_maps = []
    for c in range(N_CORES):
        b, qh = divmod(c, 2)
        # x^T [C, S] bf16, rolled so this core's q-slab is at [0:Q_LOC),
        # packed [cc, p, j, 1024] so each DMA descriptor moves 8KB
        x_c = np.roll(input_embedding[b].T, -Q_LOC * qh, axis=1).astype(
            ml_dtypes.bfloat16
        )
        x_c = x_c.reshape(4, 128, 4, 1024).transpose(2, 1, 0, 3)
        # mask^T slab [S(k), Q_LOC(q)] rolled along k, packed so group g's
        # partition p holds k-rows {g*512 + t*128 + p} (8KB contiguous)
        m_c = np.roll(mask[Q_LOC * qh : Q_LOC * (qh + 1), :].T, -Q_LOC * qh, axis=0)
        m_p = (
            m_c.astype(np.int8)
            .reshape(N_MG, 4, 128, Q_LOC)
            .transpose(0, 2, 1, 3)
            .reshape(N_MG, 128, 4 * Q_LOC)
        )
        in_maps.append(
            {
                "xt": np.ascontiguousarray(x_c),
                "maskp": np.ascontiguousarray(m_p),
                **w,
            }
        )
    return in_maps


def _gather(results):
    out = np.empty((B, S, D), dtype=np.float32)
    for c in range(N_CORES):
        b, qh = divmod(c, 2)
        out[b, Q_LOC * qh : Q_LOC * (qh + 1), :] = results[c]["out"]
    return out


def kernel(input_embedding, mask, Wq, bq, Wk, bk, Wv, bv):
    nc = build_kernel()
    in_maps = _shard_inputs(input_embedding, mask, Wq, bq, Wk, bk, Wv, bv)
    res = run_bass_kernel_spmd(nc, in_maps, list(range(N_CORES)))
    return _gather(res.results)


# revision 45
# speedup vs baseline: 1.0079x; 1.0079x over previous
"""Distributed masked-attention kernel for one TRN2 chip (8 NeuronCores).

Problem: B=4, S=4096, IN=512, D=64 attention with a [S,S] int32 score mask
(masked scores replaced by 1e-6 *before* softmax, so masked probs are
exp(1e-6)/Z ~= 1/Z, NOT zero).

Sharding (8 cores): core c = b*2 + qh -> batch b in {0..3}, query rows
[2048*qh, 2048*(qh+1)). Inputs are rolled along S so the core's own query
slab is at rows [0:2048) (attention's k-sum is permutation invariant) ->
all 8 cores run the IDENTICAL graph (SPMD).

The steady-state loop is DVE-bound on TRN2: the mask multiply must read the
f32 PSUM scores at 1x (bf16 PSUM matmul output that would enable the DVE
2x_1p mode is TRN3-only), so 64 x [128,1024] tensor_tensor ops ~= 78us is
the floor. Everything else is arranged to hide under it:
  PE:  S^T = (K^T block)^T @ Q^T  (bf16, Q^T zero-padded to 128 partitions
       -> full-array contraction keeps the PE HAM at 2.4GHz)
  DVE: sm = S^T * mask -> SBUF bf16 (1x from f32 PSUM: the pacing stream).
       Mid-loop KV bias-adds/vaug copies also ride DVE (prompt) instead of
       parking behind a 3.7us exp on the scalar queue.
  ACT: P = exp(0.125*sm) in FD=4096 groups (2 k-tiles x 2 q-halves per
       ACTIVATE) -> ~59us total, never paces the loop.
  PE:  O^T[65, q] += V_aug^T @ P  (V_aug = [V | 1]: ones column emits the
       softmax denominator for free)

Ramp: every DMA rides the sync ring in strict deadline order (the sem-lane
reuse parking then lands on the idle sync queue, not the scalar engine);
~7 warm-up matmuls on scratch data unthrottle the PE HAM before the real
projections; the exp table loads at t~7us via a dummy ACTIVATE; the qh1
Q-projection is interleaved between the first k-tile's score halves.

Epilogue per q-half: two 512-col sub-blocks, each PSUM->SBUF copy (ACT),
4 PE transposes, strided reciprocal of the denominator row (DVE), divides
split DVE/ACT, store -- sub-block 0 starts as soon as its own PV
accumulation chain stops, pipelining the tail across engines.
"""

import sys

if "/opt/trn_rl_repo" not in sys.path:
    sys.path.insert(0, "/opt/trn_rl_repo")

from contextlib import ExitStack

import numpy as np

import concourse.bass as bass
import concourse.bacc as bacc
import concourse.mybir as mybir
import concourse.tile as tile
from concourse.bass_utils import run_bass_kernel_spmd
from concourse.masks import make_identity

ts = bass.ts
ds = bass.ds

N_CORES = 8
B, S, C, D = 4, 4096, 512, 64
Q_LOC = 2048       # query rows per core
QH = 1024          # one q-half (pipeline stream) width
N_KT = S // 128    # 32 k-tiles of 128
QC = 512           # matmul moving chunk
N_MG = 8           # mask DMA groups (4 k-tiles each)

F32 = mybir.dt.float32
BF16 = mybir.dt.bfloat16
I8 = mybir.dt.int8
AF = mybir.ActivationFunctionType
ALU = mybir.AluOpType

# exp groups: [kt0] alone (early ACT start), then pairs, then singles at the
# end (short tail + lets the PV lag taper off); kt31 is handled separately.
GROUPS = [[0]] + [[1 + 2 * i, 2 + 2 * i] for i in range(14)] + [[29], [30]]
# remaining KV chunk-halves (c,h): the projection (4 matmuls + bias) and
# the V-transposes (+vaug copy) are emitted at ADJACENT groups' starts,
# batched in pairs -- two smaller insertions stall the in-order PE queue
# (and so the DVE mask-multiply stream) less than one big one, and the
# deferral is safe because kvt is stable SBUF and vaug is only read by the
# lagged PV.  Measured: split 122.8us vs fused 123.9 vs singles 123.8+.
KV_SCHED_P = {
    1: [(0, 1), (1, 0)],
    5: [(1, 1), (2, 0)],
    9: [(2, 1), (3, 0)],
    12: [(3, 1)],
}
KV_SCHED_T = {
    2: [(0, 1), (1, 0)],
    6: [(1, 1), (2, 0)],
    10: [(2, 1), (3, 0)],
    13: [(3, 1)],
}


def build_kernel() -> bacc.Bacc:
    nc = bacc.Bacc(None, target_bir_lowering=False, debug=False)

    xt_ext = nc.declare_dram_parameter("xt", [4, 128, 4, 1024], BF16, isOutput=False)
    mt_ext = nc.declare_dram_parameter("maskp", [N_MG, 128, 4 * Q_LOC], I8, isOutput=False)
    # packed weights: [Wk | Wv | Wq] along the last dim
    w_ext = nc.declare_dram_parameter("wqkv", [128, 4, 3 * D], BF16, isOutput=False)
    # packed biases: col0 = [bk; bv], col1 = [bq; 0]
    b_ext = nc.declare_dram_parameter("biases", [128, 2], F32, isOutput=False)
    out_ext = nc.declare_dram_parameter("out", [Q_LOC, D], F32, isOutput=True)

    with tile.TileContext(nc) as tc, ExitStack() as ctx:
        # ---------------- pools ----------------
        persist = ctx.enter_context(tc.tile_pool(name="persist", bufs=1))
        sm_pool = ctx.enter_context(tc.tile_pool(name="sm", bufs=4))
        pt_pool = ctx.enter_context(tc.tile_pool(name="pt", bufs=5))
        epi = ctx.enter_context(tc.tile_pool(name="epi", bufs=1))
        epi2 = ctx.enter_context(tc.tile_pool(name="epi2", bufs=2))
        psum_s = ctx.enter_context(
            tc.tile_pool(name="psum_s", bufs=2, space=bass.MemorySpace.PSUM)
        )
        psum_o = ctx.enter_context(
            tc.tile_pool(name="psum_o", bufs=1, space=bass.MemorySpace.PSUM)
        )

        # ---------------- persistent tiles ----------------
        wt = persist.tile([128, 4, 3 * D], BF16)
        biasT = persist.tile([128, 2], F32)
        xb = persist.tile([128, 4, 4, 1024], BF16, name="xb", tag="xb")
        mg = [persist.tile([128, 4, Q_LOC], I8, name=f"mg{g}", tag=f"mg{g}") for g in range(N_MG)]
        kvt = persist.tile([128, S], BF16, name="kvt", tag="kvt")
        qt_t = persist.tile([128, Q_LOC], BF16, name="qt", tag="qt")
        vaug = persist.tile([128, N_KT, D + 1], BF16, name="va", tag="va")
        scr_in = persist.tile([128, 8], BF16, name="scr_in", tag="scr")
        scr_out = persist.tile([128, 8], BF16, name="scr_out", tag="scr2")
        # init memsets on the (ramp-idle) DVE
        nc.vector.memset(qt_t[D:128, :], 0.0)
        nc.vector.memset(vaug[:, :, D : D + 1], 1.0)
        nc.vector.memset(scr_in[:], 0.0)
        # scratch source for PE warm-up matmuls (HAM needs ~3.4us of activity
        # before it unthrottles 1.2 -> 2.4 GHz)
        nc.vector.memset(kvt[:, 0:QC], 0.0)

        # Preload the exp table set right away (one-time ~2.7us); the scalar
        # queue carries no DMA triggers so this runs at t~7us.
        nc.scalar.activation(scr_out[:], scr_in[:], AF.Exp)

        # ---------------- DMAs ----------------

        # ALL DMAs ride the sync ring in strict deadline order: a single ring
        # gets full SDMA bandwidth per transfer (FIFO, no cross-ring
        # round-robin), and the unavoidable semaphore-lane-reuse parking of
        # later triggers lands on the otherwise idle sync queue instead of
        # blocking the scalar engine.
        def dma_m(g):
            nc.sync.dma_start(mg[g][:].rearrange("p t q -> p (t q)"), mt_ext[g])

        def dma_x(cc):
            # x host-packed [cc, p, j, 1024]: each partition's chunk slice is
            # 8KB contiguous = one descriptor (full DMA rate; the column-
            # sliced view was descriptor-rate-bound at ~166GB/s)
            nc.sync.dma_start(
                xb[:, cc, :, :].rearrange("p j s -> p (j s)"), xt_ext[cc]
            )

        dma_x(0)
        nc.sync.dma_start(wt[:], w_ext[:])
        nc.sync.dma_start(biasT[:], b_ext[:])
        dma_m(0)
        dma_x(1)
        dma_m(1)
        dma_x(2)
        dma_x(3)
        for g in range(2, N_MG):
            dma_m(g)

        # ---------------- constants ----------------
        ident_f = persist.tile([128, 128], F32)
        make_identity(nc, ident_f[:])
        ident_b = persist.tile([128, 128], BF16)
        make_identity(nc, ident_b[:])

        def emit_kv_proj(c: int, h: int, on_dve: bool = False):
            # on_dve: mid-loop the bias-apply/copy run on the (bottleneck but
            # prompt) DVE queue -- on ACT they'd sit behind a 3.7us exp and
            # starve the shared PSUM pool's next score tile.
            kv_ps = psum_s.tile([128, QC], F32, name="kvps", tag="ps")
            for j in range(4):
                nc.tensor.matmul(
                    kv_ps[:],
                    wt[:, j, 0 : 2 * D],
                    xb[:, c, j, ds(h * QC, QC)],
                    start=(j == 0),
                    stop=(j == 3),
                )
            if on_dve:
                nc.vector.tensor_scalar(
                    kvt[:, ds(c * 1024 + h * QC, QC)], kv_ps[:],
                    biasT[:, 0:1], None, op0=ALU.add,
                )
            else:
                nc.scalar.activation(
                    kvt[:, ds(c * 1024 + h * QC, QC)], kv_ps[:], AF.Identity,
                    bias=biasT[:, 0:1],
                )

        def emit_kv_tran(c: int, h: int, on_dve: bool = False):
            # V^T -> vaug transposes, deferred one group after the projection
            # (kvt is stable SBUF; vaug[kt] is only read by the lagged PV)
            vp = psum_s.tile([128, 4, D], BF16, name="vp", tag="ps")
            kt0 = 8 * c + 4 * h
            for u in range(4):
                nc.tensor.transpose(
                    vp[:, u, :],
                    kvt[D : 2 * D, ts(kt0 + u, 128)],
                    ident_b[D : 2 * D, D : 2 * D],
                )
            if on_dve:
                nc.vector.tensor_copy(vaug[:, kt0 : kt0 + 4, 0:D], vp[:])
            else:
                nc.scalar.copy(vaug[:, kt0 : kt0 + 4, 0:D], vp[:])

        def emit_kv_half(c: int, h: int, on_dve: bool = False):
            emit_kv_proj(c, h, on_dve)
            emit_kv_tran(c, h, on_dve)

        def emit_q(qh: int):
            q_ps = psum_s.tile([D, QH], F32, name="qps", tag="ps")
            for h in range(QH // QC):
                for j in range(4):
                    nc.tensor.matmul(
                        q_ps[:, ts(h, QC)],
                        wt[:, j, 2 * D : 3 * D],
                        xb[:, qh, j, ds(h * QC, QC)],
                        start=(j == 0),
                        stop=(j == 3),
                    )
            nc.scalar.activation(
                qt_t[0:D, ts(qh, QH)], q_ps[:], AF.Identity, bias=biasT[0:D, 1:2]
            )

        def emit_scores_tt(kt: int, qh: int, sm_q):
            """scores for (kt, qh) -> PSUM f32 -> masked bf16 into sm_q."""
            st = psum_s.tile([128, QH], F32, name="st", tag="ps")
            for qc in range(QH // QC):
                nc.tensor.matmul(
                    st[:, ts(qc, QC)],
                    kvt[:, ts(kt, 128)],
                    qt_t[:, ds(qh * QH + qc * QC, QC)],
                    start=True,
                    stop=True,
                )
            mk = mg[kt // 4][:, kt % 4, ts(qh, QH)]
            nc.vector.tensor_tensor(out=sm_q, in0=st[:], in1=mk, op=ALU.mult)

        def emit_pv(qh, kt, ot, pt_q, first, last):
            for qc in range(QH // QC):
                nc.tensor.matmul(
                    ot[:, ds(qh * QH + qc * QC, QC)],
                    vaug[:, kt, :],
                    pt_q[:, ts(qc, QC)],
                    start=first,
                    stop=last,
                )

        def emit_epilogue_half(ot, half):
            # one q-half, processed as two 512-col sub-blocks that pipeline
            # across ACT (copy) / PE (transpose) / DVE (recip, divide): the
            # first sub-block only depends on its own PV accumulation chain,
            # so it starts while the second is still accumulating.
            oext = out_ext[:].rearrange("(hf qt p) d -> hf p qt d", hf=2, p=128)
            rcp = epi2.tile([128, 16], F32, tag="rcp")
            for sub in range(2):
                ots = epi.tile([D + 1, QC], F32, tag=f"ots{half}{sub}")
                nc.scalar.copy(ots[:], ot[:, ds(half * QH + sub * QC, QC)])
                of = epi2.tile([128, 4, D], F32, tag=f"of{half}{sub}")
                op8 = psum_s.tile([128, 4, 128], F32, name="op8", tag="ps")
                for i in range(4):
                    nc.tensor.transpose(
                        op8[:, i, 0 : D + 1], ots[:, ts(i, 128)],
                        ident_f[0 : D + 1, 0 : D + 1],
                    )
                r0 = 8 * half + 4 * sub
                nc.vector.reciprocal(
                    rcp[:, ds(r0, 4)], op8[:, :, D : D + 1].rearrange("p t o -> p (t o)")
                )
                for i in range(4):
                    # divides split across DVE and ACT -- both idle in the tail
                    r = rcp[:, r0 + i : r0 + i + 1]
                    if i % 2 == 0:
                        nc.vector.tensor_scalar(
                            of[:, i, :], op8[:, i, 0:D], r, None, op0=ALU.mult
                        )
                    else:
                        nc.scalar.activation(
                            of[:, i, :], op8[:, i, 0:D], AF.Copy, scale=r
                        )
                nc.sync.dma_start(oext[half][:, ds(4 * sub, 4), :], of[:])

        # ---------------- emission ----------------
        ot = psum_o.tile([D + 1, Q_LOC], F32, name="ot", tag="ot")
        # PE warm-up: dummy matmuls on scratch data while x streams in, so
        # the HAM unthrottles before the real projections start.
        wrm = psum_s.tile([128, QC], F32, name="wrm", tag="ps")
        for _ in range(7):
            nc.tensor.matmul(wrm[:], ident_b[:], kvt[:, 0:QC], start=True, stop=True)
        emit_kv_half(0, 0)
        emit_q(0)

        pv_queue = []  # (kt, pt_qh0, pt_qh1)

        def drain_pv(keep):
            # pop entries while more than `keep` remain queued.  The kept
            # backlog is the PV lag: a popped entry's exp must be long done
            # or the in-order PE queue head-of-line blocks on it.
            while len(pv_queue) > keep:
                kt, pa, pb = pv_queue.pop(0)
                emit_pv(0, kt, ot, pa, kt == 0, False)
                emit_pv(1, kt, ot, pb, kt == 0, False)

        def emit_exp(sm, pt):
            nc.scalar.activation(
                pt[:].rearrange("p a q -> p (a q)"),
                sm[:].rearrange("p a q -> p (a q)"),
                AF.Exp,
                scale=0.125,
            )

        # group 0 = [kt0]: interleave the qh1 Q-projection between the two
        # score halves so the first TT fires as soon as q(0)+mask g0 land.
        sm = sm_pool.tile([128, 2, QH], BF16, tag="sm")
        emit_scores_tt(0, 0, sm[:, 0, :])
        emit_q(1)
        emit_scores_tt(0, 1, sm[:, 1, :])
        pt = pt_pool.tile([128, 2, QH], BF16, tag="pt")
        emit_exp(sm, pt)
        pv_queue.append((0, pt[:, 0, :], pt[:, 1, :]))

        n_groups = len(GROUPS) + 1  # +1: kt31 handled below
        for gi, kts in list(enumerate(GROUPS))[1:]:
            for ch in KV_SCHED_P.get(gi, []):
                emit_kv_proj(*ch, on_dve=True)
            for ch in KV_SCHED_T.get(gi, []):
                emit_kv_tran(*ch, on_dve=True)
            # taper the lag near the end so the tail flush stays short
            keep = 5 if gi < n_groups - 6 else (3 if gi < n_groups - 4 else (2 if gi < n_groups - 2 else 1))
            nq = 2 * len(kts)
            sm = sm_pool.tile([128, nq, QH], BF16, tag="sm")
            for i, kt in enumerate(kts):
                for qh in range(2):
                    emit_scores_tt(kt, qh, sm[:, 2 * i + qh, :])
                drain_pv(keep)
            pt = pt_pool.tile([128, nq, QH], BF16, tag="pt")
            emit_exp(sm, pt)
            for i, kt in enumerate(kts):
                pv_queue.append((kt, pt[:, 2 * i, :], pt[:, 2 * i + 1, :]))

        # kt31: per-qh FD=1024 exps so the qh0 PV + epilogue start ~1us
        # earlier, and the PV flush + both epilogue halves pipeline across
        # PE/ACT/DVE.
        sm31 = sm_pool.tile([128, 2, QH], BF16, tag="sm")
        pt31 = pt_pool.tile([128, 2, QH], BF16, tag="pt")
        emit_scores_tt(31, 0, sm31[:, 0, :])
        nc.scalar.activation(pt31[:, 0, :], sm31[:, 0, :], AF.Exp, scale=0.125)
        emit_scores_tt(31, 1, sm31[:, 1, :])
        nc.scalar.activation(pt31[:, 1, :], sm31[:, 1, :], AF.Exp, scale=0.125)
        drain_pv(0)
        emit_pv(0, 31, ot, pt31[:, 0, :], False, True)
        emit_pv(1, 31, ot, pt31[:, 1, :], False, True)
        emit_epilogue_half(ot, 0)
        emit_epilogue_half(ot, 1)

    nc.compile()
    return nc


def _shard_inputs(input_embedding, mask, Wq, bq, Wk, bk, Wv, bv):
    import ml_dtypes

    input_embedding = np.asarray(input_embedding, dtype=np.float32)
    mask = np.asarray(mask, dtype=np.int32)

    def pack_w(w):
        return np.ascontiguousarray(
            np.asarray(w, np.float32).reshape(4, 128, -1).transpose(1, 0, 2)
        ).astype(ml_dtypes.bfloat16)

    wqkv = np.concatenate([pack_w(Wk), pack_w(Wv), pack_w(Wq)], axis=2)
    biases = np.zeros((128, 2), np.float32)
    biases[:, 0] = np.concatenate([np.asarray(bk, np.float32), np.asarray(bv, np.float32)])
    biases[0:D, 1] = np.asarray(bq, np.float32)
    w = {
        "wqkv": np.ascontiguousarray(wqkv),
        "biases": np.ascontiguousarray(biases),
    }
    in_maps = []
    for c in range(N_CORES):
        b, qh = divmod(c, 2)
        # x^T [C, S] bf16, rolled so this core's q-slab is at [0:Q_LOC),
        # packed [cc, p, j, 1024] so each DMA descriptor moves 8KB
        x_c = np.roll(input_embedding[b].T, -Q_LOC * qh, axis=1).astype(
            ml_dtypes.bfloat16
        )
        x_c = x_c.reshape(4, 128, 4, 1024).transpose(2, 1, 0, 3)
        # mask^T slab [S(k), Q_LOC(q)] rolled along k, packed so group g's
        # partition p holds k-rows {g*512 + t*128 + p} (8KB contiguous)
        m_c = np.roll(mask[Q_LOC * qh : Q_LOC * (qh + 1), :].T, -Q_LOC * qh, axis=0)
        m_p = (
            m_c.astype(np.int8)
            .reshape(N_MG, 4, 128, Q_LOC)
            .transpose(0, 2, 1, 3)
            .reshape(N_MG, 128, 4 * Q_LOC)
        )
        in_maps.append(
            {
                "xt": np.ascontiguousarray(x_c),
                "maskp": np.ascontiguousarray(m_p),
                **w,
            }
        )
    return in_maps


def _gather(results):
    out = np.empty((B, S, D), dtype=np.float32)
    for c in range(N_CORES):
        b, qh = divmod(c, 2)
        out[b, Q_LOC * qh : Q_LOC * (qh + 1), :] = results[c]["out"]
    return out


def kernel(input_embedding, mask, Wq, bq, Wk, bk, Wv, bv):
    nc = build_kernel()
    in_maps = _shard_inputs(input_embedding, mask, Wq, bq, Wk, bk, Wv, bv)
    res = run_bass_kernel_spmd(nc, in_maps, list(range(N_CORES)))
    return _gather(res.results)


# revision 46
# speedup vs baseline: 1.0092x; 1.0013x over previous
"""Distributed masked-attention kernel for one TRN2 chip (8 NeuronCores).

Problem: B=4, S=4096, IN=512, D=64 attention with a [S,S] int32 score mask
(masked scores replaced by 1e-6 *before* softmax, so masked probs are
exp(1e-6)/Z ~= 1/Z, NOT zero).

Sharding (8 cores): core c = b*2 + qh -> batch b in {0..3}, query rows
[2048*qh, 2048*(qh+1)). Inputs are rolled along S so the core's own query
slab is at rows [0:2048) (attention's k-sum is permutation invariant) ->
all 8 cores run the IDENTICAL graph (SPMD).

The steady-state loop is DVE-bound on TRN2: the mask multiply must read the
f32 PSUM scores at 1x (bf16 PSUM matmul output that would enable the DVE
2x_1p mode is TRN3-only), so 64 x [128,1024] tensor_tensor ops ~= 78us is
the floor. Everything else is arranged to hide under it:
  PE:  S^T = (K^T block)^T @ Q^T  (bf16, Q^T zero-padded to 128 partitions
       -> full-array contraction keeps the PE HAM at 2.4GHz)
  DVE: sm = S^T * mask -> SBUF bf16 (1x from f32 PSUM: the pacing stream).
       Mid-loop KV bias-adds/vaug copies also ride DVE (prompt) instead of
       parking behind a 3.7us exp on the scalar queue.
  ACT: P = exp(0.125*sm) in FD=4096 groups (2 k-tiles x 2 q-halves per
       ACTIVATE) -> ~59us total, never paces the loop.
  PE:  O^T[65, q] += V_aug^T @ P  (V_aug = [V | 1]: ones column emits the
       softmax denominator for free)

Ramp: every DMA rides the sync ring in strict deadline order (the sem-lane
reuse parking then lands on the idle sync queue, not the scalar engine);
~7 warm-up matmuls on scratch data unthrottle the PE HAM before the real
projections; the exp table loads at t~7us via a dummy ACTIVATE; the qh1
Q-projection is interleaved between the first k-tile's score halves.

Epilogue per q-half: two 512-col sub-blocks, each PSUM->SBUF copy (ACT),
4 PE transposes, strided reciprocal of the denominator row (DVE), divides
split DVE/ACT, store -- sub-block 0 starts as soon as its own PV
accumulation chain stops, pipelining the tail across engines.
"""

import sys

if "/opt/trn_rl_repo" not in sys.path:
    sys.path.insert(0, "/opt/trn_rl_repo")

from contextlib import ExitStack

import numpy as np

import concourse.bass as bass
import concourse.bacc as bacc
import concourse.mybir as mybir
import concourse.tile as tile
from concourse.bass_utils import run_bass_kernel_spmd
from concourse.masks import make_identity

ts = bass.ts
ds = bass.ds

N_CORES = 8
B, S, C, D = 4, 4096, 512, 64
Q_LOC = 2048       # query rows per core
QH = 1024          # one q-half (pipeline stream) width
N_KT = S // 128    # 32 k-tiles of 128
QC = 512           # matmul moving chunk
N_MG = 8           # mask DMA groups (4 k-tiles each)

F32 = mybir.dt.float32
BF16 = mybir.dt.bfloat16
I8 = mybir.dt.int8
AF = mybir.ActivationFunctionType
ALU = mybir.AluOpType

# exp groups: [kt0] alone (early ACT start), then pairs, then singles at the
# end (short tail + lets the PV lag taper off); kt31 is handled separately.
GROUPS = [[0]] + [[1 + 2 * i, 2 + 2 * i] for i in range(14)] + [[29], [30]]
# remaining KV chunk-halves (c,h): the projection (4 matmuls + bias) and
# the V-transposes (+vaug copy) are emitted at ADJACENT groups' starts,
# batched in pairs -- two smaller insertions stall the in-order PE queue
# (and so the DVE mask-multiply stream) less than one big one, and the
# deferral is safe because kvt is stable SBUF and vaug is only read by the
# lagged PV.  Measured: split 122.8us vs fused 123.9 vs singles 123.8+.
KV_SCHED_P = {
    1: [(0, 1), (1, 0)],
    5: [(1, 1), (2, 0)],
    9: [(2, 1), (3, 0)],
    12: [(3, 1)],
}
KV_SCHED_T = {
    2: [(0, 1), (1, 0)],
    6: [(1, 1), (2, 0)],
    10: [(2, 1), (3, 0)],
    13: [(3, 1)],
}


def build_kernel() -> bacc.Bacc:
    nc = bacc.Bacc(None, target_bir_lowering=False, debug=False)

    xt_ext = nc.declare_dram_parameter("xt", [4, 128, 4, 1024], BF16, isOutput=False)
    mt_ext = nc.declare_dram_parameter("maskp", [N_MG, 128, 4 * Q_LOC], I8, isOutput=False)
    # packed weights: [Wk | Wv | Wq] along the last dim
    w_ext = nc.declare_dram_parameter("wqkv", [128, 4, 3 * D], BF16, isOutput=False)
    # packed biases: col0 = [bk; bv], col1 = [bq; 0]
    b_ext = nc.declare_dram_parameter("biases", [128, 2], F32, isOutput=False)
    out_ext = nc.declare_dram_parameter("out", [Q_LOC, D], F32, isOutput=True)

    with tile.TileContext(nc) as tc, ExitStack() as ctx:
        # ---------------- pools ----------------
        persist = ctx.enter_context(tc.tile_pool(name="persist", bufs=1))
        sm_pool = ctx.enter_context(tc.tile_pool(name="sm", bufs=4))
        pt_pool = ctx.enter_context(tc.tile_pool(name="pt", bufs=5))
        epi = ctx.enter_context(tc.tile_pool(name="epi", bufs=1))
        epi2 = ctx.enter_context(tc.tile_pool(name="epi2", bufs=2))
        psum_s = ctx.enter_context(
            tc.tile_pool(name="psum_s", bufs=2, space=bass.MemorySpace.PSUM)
        )
        psum_o = ctx.enter_context(
            tc.tile_pool(name="psum_o", bufs=1, space=bass.MemorySpace.PSUM)
        )

        # ---------------- persistent tiles ----------------
        wt = persist.tile([128, 4, 3 * D], BF16)
        biasT = persist.tile([128, 2], F32)
        xb = persist.tile([128, 4, 4, 1024], BF16, name="xb", tag="xb")
        mg = [persist.tile([128, 4, Q_LOC], I8, name=f"mg{g}", tag=f"mg{g}") for g in range(N_MG)]
        kvt = persist.tile([128, S], BF16, name="kvt", tag="kvt")
        qt_t = persist.tile([128, Q_LOC], BF16, name="qt", tag="qt")
        vaug = persist.tile([128, N_KT, D + 1], BF16, name="va", tag="va")
        scr_in = persist.tile([128, 8], BF16, name="scr_in", tag="scr")
        scr_out = persist.tile([128, 8], BF16, name="scr_out", tag="scr2")
        # init memsets on the (ramp-idle) DVE
        nc.vector.memset(qt_t[D:128, :], 0.0)
        nc.vector.memset(vaug[:, :, D : D + 1], 1.0)
        nc.vector.memset(scr_in[:], 0.0)
        # scratch source for PE warm-up matmuls (HAM needs ~3.4us of activity
        # before it unthrottles 1.2 -> 2.4 GHz)
        nc.vector.memset(kvt[:, 0:QC], 0.0)

        # Preload the exp table set right away (one-time ~2.7us); the scalar
        # queue carries no DMA triggers so this runs at t~7us.
        nc.scalar.activation(scr_out[:], scr_in[:], AF.Exp)

        # ---------------- DMAs ----------------

        # ALL DMAs ride the sync ring in strict deadline order: a single ring
        # gets full SDMA bandwidth per transfer (FIFO, no cross-ring
        # round-robin), and the unavoidable semaphore-lane-reuse parking of
        # later triggers lands on the otherwise idle sync queue instead of
        # blocking the scalar engine.
        def dma_m(g):
            nc.sync.dma_start(mg[g][:].rearrange("p t q -> p (t q)"), mt_ext[g])

        def dma_x(cc):
            # x host-packed [cc, p, j, 1024]: each partition's chunk slice is
            # 8KB contiguous = one descriptor (full DMA rate; the column-
            # sliced view was descriptor-rate-bound at ~166GB/s)
            nc.sync.dma_start(
                xb[:, cc, :, :].rearrange("p j s -> p (j s)"), xt_ext[cc]
            )

        dma_x(0)
        nc.sync.dma_start(wt[:], w_ext[:])
        nc.sync.dma_start(biasT[:], b_ext[:])
        dma_m(0)
        dma_x(1)
        dma_m(1)
        dma_x(2)
        dma_x(3)
        for g in range(2, N_MG):
            dma_m(g)

        # ---------------- constants ----------------
        ident_f = persist.tile([128, 128], F32)
        make_identity(nc, ident_f[:])
        ident_b = persist.tile([128, 128], BF16)
        make_identity(nc, ident_b[:])

        def emit_kv_proj(c: int, h: int, on_dve: bool = False):
            # on_dve: mid-loop the bias-apply/copy run on the (bottleneck but
            # prompt) DVE queue -- on ACT they'd sit behind a 3.7us exp and
            # starve the shared PSUM pool's next score tile.
            kv_ps = psum_s.tile([128, QC], F32, name="kvps", tag="ps")
            for j in range(4):
                nc.tensor.matmul(
                    kv_ps[:],
                    wt[:, j, 0 : 2 * D],
                    xb[:, c, j, ds(h * QC, QC)],
                    start=(j == 0),
                    stop=(j == 3),
                )
            if on_dve:
                nc.vector.tensor_scalar(
                    kvt[:, ds(c * 1024 + h * QC, QC)], kv_ps[:],
                    biasT[:, 0:1], None, op0=ALU.add,
                )
            else:
                nc.scalar.activation(
                    kvt[:, ds(c * 1024 + h * QC, QC)], kv_ps[:], AF.Identity,
                    bias=biasT[:, 0:1],
                )

        def emit_kv_tran(c: int, h: int, on_dve: bool = False):
            # V^T -> vaug transposes, deferred one group after the projection
            # (kvt is stable SBUF; vaug[kt] is only read by the lagged PV)
            vp = psum_s.tile([128, 4, D], BF16, name="vp", tag="ps")
            kt0 = 8 * c + 4 * h
            for u in range(4):
                nc.tensor.transpose(
                    vp[:, u, :],
                    kvt[D : 2 * D, ts(kt0 + u, 128)],
                    ident_b[D : 2 * D, D : 2 * D],
                )
            if on_dve:
                nc.vector.tensor_copy(vaug[:, kt0 : kt0 + 4, 0:D], vp[:])
            else:
                nc.scalar.copy(vaug[:, kt0 : kt0 + 4, 0:D], vp[:])

        def emit_kv_half(c: int, h: int, on_dve: bool = False):
            emit_kv_proj(c, h, on_dve)
            emit_kv_tran(c, h, on_dve)

        def emit_q(qh: int):
            q_ps = psum_s.tile([D, QH], F32, name="qps", tag="ps")
            for h in range(QH // QC):
                for j in range(4):
                    nc.tensor.matmul(
                        q_ps[:, ts(h, QC)],
                        wt[:, j, 2 * D : 3 * D],
                        xb[:, qh, j, ds(h * QC, QC)],
                        start=(j == 0),
                        stop=(j == 3),
                    )
            nc.scalar.activation(
                qt_t[0:D, ts(qh, QH)], q_ps[:], AF.Identity, bias=biasT[0:D, 1:2]
            )

        def emit_scores_tt(kt: int, qh: int, sm_q):
            """scores for (kt, qh) -> PSUM f32 -> masked bf16 into sm_q."""
            st = psum_s.tile([128, QH], F32, name="st", tag="ps")
            for qc in range(QH // QC):
                nc.tensor.matmul(
                    st[:, ts(qc, QC)],
                    kvt[:, ts(kt, 128)],
                    qt_t[:, ds(qh * QH + qc * QC, QC)],
                    start=True,
                    stop=True,
                )
            mk = mg[kt // 4][:, kt % 4, ts(qh, QH)]
            nc.vector.tensor_tensor(out=sm_q, in0=st[:], in1=mk, op=ALU.mult)

        def emit_pv(qh, kt, ot, pt_q, first, last):
            for qc in range(QH // QC):
                nc.tensor.matmul(
                    ot[:, ds(qh * QH + qc * QC, QC)],
                    vaug[:, kt, :],
                    pt_q[:, ts(qc, QC)],
                    start=first,
                    stop=last,
                )

        def emit_epilogue_half(ot, half):
            # one q-half, processed as two 512-col sub-blocks that pipeline
            # across ACT (copy) / PE (transpose) / DVE (recip, divide): the
            # first sub-block only depends on its own PV accumulation chain,
            # so it starts while the second is still accumulating.
            oext = out_ext[:].rearrange("(hf qt p) d -> hf p qt d", hf=2, p=128)
            rcp = epi2.tile([128, 16], F32, tag="rcp")
            for sub in range(2):
                ots = epi.tile([D + 1, QC], F32, tag=f"ots{half}{sub}")
                nc.scalar.copy(ots[:], ot[:, ds(half * QH + sub * QC, QC)])
                of = epi2.tile([128, 4, D], F32, tag=f"of{half}{sub}")
                op8 = psum_s.tile([128, 4, 128], F32, name="op8", tag="ps")
                for i in range(4):
                    nc.tensor.transpose(
                        op8[:, i, 0 : D + 1], ots[:, ts(i, 128)],
                        ident_f[0 : D + 1, 0 : D + 1],
                    )
                r0 = 8 * half + 4 * sub
                nc.vector.reciprocal(
                    rcp[:, ds(r0, 4)], op8[:, :, D : D + 1].rearrange("p t o -> p (t o)")
                )
                for i in range(4):
                    # divides split across DVE and ACT -- both idle in the tail
                    r = rcp[:, r0 + i : r0 + i + 1]
                    if i % 2 == 0:
                        nc.vector.tensor_scalar(
                            of[:, i, :], op8[:, i, 0:D], r, None, op0=ALU.mult
                        )
                    else:
                        nc.scalar.activation(
                            of[:, i, :], op8[:, i, 0:D], AF.Copy, scale=r
                        )
                nc.sync.dma_start(oext[half][:, ds(4 * sub, 4), :], of[:])

        # ---------------- emission ----------------
        ot = psum_o.tile([D + 1, Q_LOC], F32, name="ot", tag="ot")
        # PE warm-up: dummy matmuls on scratch data while x streams in, so
        # the HAM unthrottles before the real projections start.
        wrm = psum_s.tile([128, QC], F32, name="wrm", tag="ps")
        for _ in range(7):
            nc.tensor.matmul(wrm[:], ident_b[:], kvt[:, 0:QC], start=True, stop=True)
        emit_kv_half(0, 0)
        emit_q(0)

        pv_queue = []  # (kt, pt_qh0, pt_qh1)

        def drain_pv(keep):
            # pop entries while more than `keep` remain queued.  The kept
            # backlog is the PV lag: a popped entry's exp must be long done
            # or the in-order PE queue head-of-line blocks on it.
            while len(pv_queue) > keep:
                kt, pa, pb = pv_queue.pop(0)
                emit_pv(0, kt, ot, pa, kt == 0, False)
                emit_pv(1, kt, ot, pb, kt == 0, False)

        def emit_exp(sm, pt):
            nc.scalar.activation(
                pt[:].rearrange("p a q -> p (a q)"),
                sm[:].rearrange("p a q -> p (a q)"),
                AF.Exp,
                scale=0.125,
            )

        # group 0 = [kt0]: interleave the qh1 Q-projection between the two
        # score halves so the first TT fires as soon as q(0)+mask g0 land.
        sm = sm_pool.tile([128, 2, QH], BF16, tag="sm")
        emit_scores_tt(0, 0, sm[:, 0, :])
        emit_q(1)
        emit_scores_tt(0, 1, sm[:, 1, :])
        pt = pt_pool.tile([128, 2, QH], BF16, tag="pt")
        emit_exp(sm, pt)
        pv_queue.append((0, pt[:, 0, :], pt[:, 1, :]))

        n_groups = len(GROUPS) + 1  # +1: kt31 handled below
        for gi, kts in list(enumerate(GROUPS))[1:]:
            for ch in KV_SCHED_P.get(gi, []):
                emit_kv_proj(*ch, on_dve=True)
            for ch in KV_SCHED_T.get(gi, []):
                emit_kv_tran(*ch, on_dve=True)
            # taper the lag near the end so the tail flush stays short
            keep = 5 if gi < n_groups - 6 else (3 if gi < n_groups - 4 else (2 if gi < n_groups - 2 else 1))
            nq = 2 * len(kts)
            sm = sm_pool.tile([128, nq, QH], BF16, tag="sm")
            for i, kt in enumerate(kts):
                for qh in range(2):
                    emit_scores_tt(kt, qh, sm[:, 2 * i + qh, :])
                    drain_pv(keep + (1 - qh))
            pt = pt_pool.tile([128, nq, QH], BF16, tag="pt")
            emit_exp(sm, pt)
            for i, kt in enumerate(kts):
                pv_queue.append((kt, pt[:, 2 * i, :], pt[:, 2 * i + 1, :]))

        # kt31: per-qh FD=1024 exps so the qh0 PV + epilogue start ~1us
        # earlier, and the PV flush + both epilogue halves pipeline across
        # PE/ACT/DVE.
        sm31 = sm_pool.tile([128, 2, QH], BF16, tag="sm")
        pt31 = pt_pool.tile([128, 2, QH], BF16, tag="pt")
        emit_scores_tt(31, 0, sm31[:, 0, :])
        nc.scalar.activation(pt31[:, 0, :], sm31[:, 0, :], AF.Exp, scale=0.125)
        emit_scores_tt(31, 1, sm31[:, 1, :])
        nc.scalar.activation(pt31[:, 1, :], sm31[:, 1, :], AF.Exp, scale=0.125)
        drain_pv(0)
        emit_pv(0, 31, ot, pt31[:, 0, :], False, True)
        emit_pv(1, 31, ot, pt31[:, 1, :], False, True)
        emit_epilogue_half(ot, 0)
        emit_epilogue_half(ot, 1)

    nc.compile()
    return nc


def _shard_inputs(input_embedding, mask, Wq, bq, Wk, bk, Wv, bv):
    import ml_dtypes

    input_embedding = np.asarray(input_embedding, dtype=np.float32)
    mask = np.asarray(mask, dtype=np.int32)

    def pack_w(w):
        return np.ascontiguousarray(
            np.asarray(w, np.float32).reshape(4, 128, -1).transpose(1, 0, 2)
        ).astype(ml_dtypes.bfloat16)

    wqkv = np.concatenate([pack_w(Wk), pack_w(Wv), pack_w(Wq)], axis=2)
    biases = np.zeros((128, 2), np.float32)
    biases[:, 0] = np.concatenate([np.asarray(bk, np.float32), np.asarray(bv, np.float32)])
    biases[0:D, 1] = np.asarray(bq, np.float32)
    w = {
        "wqkv": np.ascontiguousarray(wqkv),
        "biases": np.ascontiguousarray(biases),
    }
    in_maps = []
    for c in range(N_CORES):
        b, qh = divmod(c, 2)
        # x^T [C, S] bf16, rolled so this core's q-slab is at [0:Q_LOC),
        # packed [cc, p, j, 1024] so each DMA descriptor moves 8KB
        x_c = np.roll(input_embedding[b].T, -Q_LOC * qh, axis=1).astype(
            ml_dtypes.bfloat16
        )
        x_c = x_c.reshape(4, 128, 4, 1024).transpose(2, 1, 0, 3)
        # mask^T slab [S(k), Q_LOC(q)] rolled along k, packed so group g's
        # partition p holds k-rows {g*512 + t*128 + p} (8KB contiguous)
        m_c = np.roll(mask[Q_LOC * qh : Q_LOC * (qh + 1), :].T, -Q_LOC * qh, axis=0)
        m_p = (
            m_c.astype(np.int8)
            .reshape(N_MG, 4, 128, Q_LOC)
            .transpose(0, 2, 1, 3)
            .reshape(N_MG, 128, 4 * Q_LOC)
        )
        in_maps.append(
            {
                "xt": np.ascontiguousarray(x_c),
                "maskp": np.ascontiguousarray(m_p),
                **w,
            }
        )
    return in_maps


def _gather(results):
    out = np.empty((B, S, D), dtype=np.float32)
    for c in range(N_CORES):
        b, qh = divmod(c, 2)
        out[b, Q_LOC * qh : Q_LOC * (qh + 1), :] = results[c]["out"]
    return out


def kernel(input_embedding, mask, Wq, bq, Wk, bk, Wv, bv):
    nc = build_kernel()
    in_maps = _shard_inputs(input_embedding, mask, Wq, bq, Wk, bk, Wv, bv)
    res = run_bass_kernel_spmd(nc, in_maps, list(range(N_CORES)))
    return _gather(res.results)
